# revision 1
# baseline (speedup 1.0000x reference)
"""Bass/Trainium2 kernel for nn_CurveGraphic2d (min-distance curve rasterizer).

kernel(**inputs) takes FULL inputs (inputs [64,4,2] f32, widths [64] f32,
aa_factors [64] f32) and returns the FULL [64,256,256] float32 canvas.

Math (per curve b, output element [b, i, j]; reference flattens its pixel
grid x-major, so row index i is the x coordinate, column j is y):

    md2    = min_s (j - sy_bs)^2 + (i - sx_bs)^2
    canvas = clip(1 - (md2/w_b^2)^(aa_b/2), 0, 1)

Softmin-by-matmul: with k_b = C / w_b^2,

    p(i,j)  = sum_s exp(-k d2_s) = sum_s exp(-k (i-sx_s)^2) * exp(-k (j-sy_s)^2)
            = sum_s U[s,i] * V[s,j]                    (rank-15 outer product)
    md2 ~= -ln(p) / k_b        (softmin; error ~ -ln(n_eff)/k, |.| <= 2.7 w^2/C)

so the whole distance field + min collapses into ONE K=30 bf16 matmul per
curve: lhsT[30,128] = U by (x-half, s), rhs[30,512] block-diagonal over
(x-half, y) with V blocks; out = one full PSUM bank [128, 512].  8 matmuls
fill all 8 banks = the whole per-core canvas [128, 4096] fp32.
C = 86 puts the fp32/bf16 exp underflow horizon (e^-87.3) right at the
clip boundary md = w: far pixels flush to p = 0 -> ln -> -inf -> canvas 0,
matching the reference's clipped zeros.  Host-side numpy sim of the full
dtype pipeline (bf16 U/V, fp32 PSUM w/ FTZ, bf16 tail) measures global
rel L2 error 3.7e-3 vs the fp64 reference (tolerance 2e-2); hardware run
matches.

The hardware Ln spline is only exact on [2^-64, 2^64] (saturates below,
garbage above) -- measured on device -- so Ln#1 prescales by 2^60 and the
offset is removed via clamp + Ln#2 bias (a const AP registered at init).

Tail over the full per-core canvas [128, 4096] (8 curves x 2 halves x 256):
    t  = Ln(p * 2^60)                   ACT, PSUM -> SBUF fp32
    t2 = clamp(t, LN_OFF-88.75, 41.56)  DVE one ts (min,max), fp32
    s2 = Ln(-t2/C + LN_OFF/C)           ACT -> bf16  ( = ln(md2/w^2) )
    u  = s2 * (aa/2)                    DVE tt, per-curve operand tile
    r  = Exp(u)                         ACT  ( = (md/w)^aa )
    o  = max(1 - r, 0)                  DVE ts x2 -> bf16 out
Ln+Exp share one ACT table set.  On the hardware cost model ACT is the
steady-state bottleneck: 3 passes x (4096+352)/1.2GHz ~= 11.1 us/iter
(CoreSim: 10.8 us/iter marginal, ~10x the 107.9 us baseline).  The
instruction count per iteration (8 LDW+MM pairs, 3 ACT, 4 DVE, 1 DMA) is
kept minimal because the bench runtime adds a large per-instruction cost.

Device decomposition: data-parallel over curves, core c owns curves
[8c, 8c+8); no cross-core communication.  Output bf16, cast on host.
"""

import numpy as np
from math import comb

H = W = 256
S = 15
B = 64
NCORES = 8
CPB = B // NCORES          # curves per core
UNITS = CPB * 2            # (curve, x-half) units per core

C_SOFT = 86.0              # softmin sharpness: k = C/w^2
FLUSH = 1.1755e-38         # fp32/bf16 min normal: pre-flush denormal U/V

# The hardware Ln spline is exact only on [2^-64, 2^64] (saturates below,
# garbage above).  p spans [2^-126, 2^4], so Ln#1 prescales by 2^61:
# t = Ln(p * 2^60) = ln p + 60 ln 2, keeping inputs in [2^-66.2, 2^64).
# The offset is removed by the clamp + Ln#2 bias.  z = LN_OFF - t.
LN_SCALE_E = 60
LN_OFF = LN_SCALE_E * float(np.log(2.0))      # 42.2804
T_CLAMP = 41.5625          # fp16-exact; z >= LN_OFF - T_CLAMP = 0.027 > 0

_prog_cache = {}


# ---------------------------------------------------------------------------
# host-side math
# ---------------------------------------------------------------------------

def _bezier_samples(inputs_np):
    """[B,S,2] float64 sample points (y, x) in pixel coords."""
    kp = inputs_np.astype(np.float64) * np.array([H, W], np.float64)
    K = kp.shape[1]
    ts = np.linspace(0.0, 1.0, S)
    k = np.arange(K)
    binom = np.array([comb(K - 1, i) for i in range(K)], np.float64)
    basis = binom * ts[:, None] ** k * (1.0 - ts[:, None]) ** (K - 1 - k)
    return np.einsum("sk,bkd->bsd", basis, kp)


def _make_core_inputs(sp, widths, aas, core):
    """Input tensors for one core (curves [8*core, 8*core+8))."""
    import ml_dtypes

    bf16 = ml_dtypes.bfloat16
    coords = np.arange(256, dtype=np.float64)
    # one K=30 matmul per curve: rows (h, s); lhsT[30, 128] = U split by
    # x-half, rhs[30, 512] block-diagonal over (h, y) with V in each block
    ut = np.zeros((2 * S, CPB * 128), np.float32)
    vt = np.zeros((2 * S, CPB * 512), np.float32)
    at = np.zeros((128, UNITS * 256), np.float32)  # aa/2 per curve block
    with np.errstate(under="ignore"):
        for cl in range(CPB):
            b = core * CPB + cl
            kb = C_SOFT / float(widths[b]) ** 2
            sy, sx = sp[b, :, 0], sp[b, :, 1]
            U = np.exp(-kb * (coords[None, :] - sx[:, None]) ** 2)  # [S,256]
            V = np.exp(-kb * (coords[None, :] - sy[:, None]) ** 2)  # [S,256]
            for h in range(2):
                ut[h * S:(h + 1) * S, cl * 128:(cl + 1) * 128] = \
                    U[:, h * 128:(h + 1) * 128]
                vt[h * S:(h + 1) * S, cl * 512 + h * 256:cl * 512 + (h + 1) * 256] = V
            at[:, cl * 512:(cl + 1) * 512] = np.float32(aas[b] / 2.0)
    ut[ut < FLUSH] = 0.0
    vt[vt < FLUSH] = 0.0
    utb = ut.astype(bf16)
    vtb = vt.astype(bf16)
    # flush any bf16 denormals produced by the rounding itself
    utb[utb.astype(np.float32) < FLUSH] = 0
    vtb[vtb.astype(np.float32) < FLUSH] = 0
    return {"ut": utb, "vt": vtb, "at": at.astype(bf16)}


# ---------------------------------------------------------------------------
# multi-wait workaround
# ---------------------------------------------------------------------------

def _split_multi_waits(nc):
    """This walrus build accepts only one sync-wait per instruction.  Hoist
    extra waits onto same-engine nops inserted just before the instruction
    (engine program order makes this semantically identical: all waits retire
    before the instruction issues)."""
    import concourse.mybir as mybir

    n = 0
    for fn in nc.m.functions:
        for bb in fn.blocks:
            insts = list(bb.instructions)
            out = []
            changed = False
            for inst in insts:
                si = inst.sync_info
                if si is not None and len(si.on_wait) > 1:
                    waits = list(si.on_wait)
                    for i, w in enumerate(waits[:-1]):
                        nop = mybir.InstNoOp(name=f"{inst.name}_xw{i}")
                        nop.engine = inst.engine
                        nop.sync_info = mybir.SyncInfo(on_wait=[w], on_update=[])
                        out.append(nop)
                        n += 1
                    inst.sync_info = mybir.SyncInfo(
                        on_wait=[waits[-1]], on_update=list(si.on_update)
                    )
                    changed = True
                out.append(inst)
            if changed:
                bb.instructions = out
    return n


# ---------------------------------------------------------------------------
# bass program (input-independent structure)
# ---------------------------------------------------------------------------

def _build_program(repeat=1, num_devices=NCORES):
    import concourse.bass as bass
    import concourse.mybir as mybir
    from concourse.tile import TileContext

    fp32 = mybir.dt.float32
    fp16 = mybir.dt.float16
    bf16 = mybir.dt.bfloat16
    A = mybir.AluOpType
    F = mybir.ActivationFunctionType

    nc = bass.Bass("TRN2", target_bir_lowering=False, debug=False,
                   num_devices=num_devices)
    bias_val = LN_OFF / C_SOFT
    _tb = nc.alloc_sbuf_tensor("const-float32-ln2bias", [128, 1], fp32)
    nc.gpsimd.memset(_tb.ap(), bias_val)
    nc.const_aps.aps[(fp32, bias_val)] = _tb.ap()
    nc.all_engine_barrier()
    ut_d = nc.dram_tensor("ut", [2 * S, CPB * 128], bf16, kind="ExternalInput")
    vt_d = nc.dram_tensor("vt", [2 * S, CPB * 512], bf16, kind="ExternalInput")
    at_d = nc.dram_tensor("at", [128, UNITS * 256], bf16, kind="ExternalInput")
    out_d = nc.dram_tensor("out", [128, UNITS * 256], bf16,
                           kind="ExternalOutput")

    with TileContext(nc) as tc:
        with (
            tc.tile_pool(name="const", bufs=1) as constp,
            tc.tile_pool(name="tail", bufs=1) as tailp,
            tc.tile_pool(name="ot", bufs=2) as otp,
            tc.psum_pool(name="psum", bufs=1) as psp,
        ):
            ut = constp.tile([2 * S, CPB * 128], bf16, tag="ut")
            nc.sync.dma_start(out=ut[:], in_=ut_d[:])
            vt = constp.tile([2 * S, CPB * 512], bf16, tag="vt")
            nc.sync.dma_start(out=vt[:], in_=vt_d[:])
            at = constp.tile([128, UNITS * 256], bf16, tag="at")
            nc.sync.dma_start(out=at[:], in_=at_d[:])

            pp = psp.tile([128, UNITS * 256], fp32, tag="pp")
            t = tailp.tile([128, UNITS * 256], fp32, tag="t")
            t2 = tailp.tile([128, UNITS * 256], fp32, tag="t2")
            s2 = tailp.tile([128, UNITS * 256], bf16, tag="s2")
            uu = tailp.tile([128, UNITS * 256], bf16, tag="uu")
            rr = tailp.tile([128, UNITS * 256], bf16, tag="rr")
            oo = otp.tile([128, UNITS * 256], bf16, tag="oo")

            def body():
                # p = sum_s U[s,i] V[s,j]; one K=30 matmul per curve = bank
                for cl in range(CPB):
                    nc.tensor.matmul(
                        pp[:, cl * 512:(cl + 1) * 512],
                        ut[:, cl * 128:(cl + 1) * 128],
                        vt[:, cl * 512:(cl + 1) * 512],
                        start=True, stop=True,
                    )
                # t = ln p + LN_OFF   (fp32)
                nc.scalar.activation(t[:], pp[:], F.Ln,
                                     scale=float(2.0 ** LN_SCALE_E))
                # t2 = clamp(t): z = LN_OFF - t2 in [0.027, 88.75]
                nc.vector.tensor_scalar(t2[:], t[:], T_CLAMP,
                                        LN_OFF - 88.75, A.min, A.max)
                # s2 = ln((LN_OFF - t2)/C) = ln(z/C) = ln(md2/w^2)
                nc.scalar.activation(s2[:], t2[:], F.Ln, scale=-1.0 / C_SOFT,
                                     bias=bias_val)
                # u = (aa/2) ln(md2/w^2)
                nc.vector.tensor_tensor(uu[:], s2[:], at[:], A.mult)
                # r = (md/w)^aa
                nc.scalar.activation(rr[:], uu[:], F.Exp)
                # o = max(1 - r, 0)
                nc.vector.tensor_scalar(oo[:], rr[:], -1.0, 1.0, A.mult, A.add)
                nc.vector.tensor_scalar_max(oo[:], oo[:], 0.0)
                nc.sync.dma_start(out=out_d[:], in_=oo[:])

            for _ in range(repeat):
                body()
    _split_multi_waits(nc)
    return nc


# ---------------------------------------------------------------------------
# public entry point
# ---------------------------------------------------------------------------

def _run(inputs, widths, aa_factors, repeat=1):
    from concourse.bass_utils import run_bass_kernel_spmd

    inputs = np.asarray(inputs, np.float32)
    widths = np.asarray(widths, np.float32)
    aa_factors = np.asarray(aa_factors, np.float32)
    assert inputs.shape == (B, 4, 2), inputs.shape

    sp = _bezier_samples(inputs)
    if repeat not in _prog_cache:
        _prog_cache[repeat] = _build_program(repeat)
    nc = _prog_cache[repeat]

    in_maps = [
        _make_core_inputs(sp, widths, aa_factors, c) for c in range(NCORES)
    ]
    res = run_bass_kernel_spmd(nc, in_maps, list(range(NCORES)))

    canvas = np.empty((B, H, W), np.float32)
    for c in range(NCORES):
        out = np.asarray(res.results[c]["out"])          # [128, 4096] fp16
        out = out.reshape(128, CPB, 2, 256)              # [i, cl, h, y]
        out = out.transpose(1, 2, 0, 3)                  # [cl, h, i, y]
        canvas[c * CPB:(c + 1) * CPB] = out.reshape(CPB, 256, 256)
    return canvas


def kernel(inputs, widths, aa_factors):
    return _run(inputs, widths, aa_factors, repeat=1)



# revision 6
# speedup vs baseline: 29.7875x; 29.7875x over previous
"""Bass/Trainium2 kernel for nn_CurveGraphic2d (min-distance curve rasterizer).

kernel(**inputs) takes FULL inputs (inputs [64,4,2] f32, widths [64] f32,
aa_factors [64] f32) and returns the FULL [64,256,256] float32 canvas.

Math (per curve b, output element [b, i, j]; reference flattens its pixel
grid x-major, so row index i is the x coordinate, column j is y):

    md2    = min_s (j - sy_bs)^2 + (i - sx_bs)^2
    canvas = clip(1 - (md2/w_b^2)^(aa_b/2), 0, 1)

Softmin-by-matmul: with k_b = C / w_b^2,

    p(i,j)  = sum_s exp(-k d2_s) = sum_s exp(-k (i-sx_s)^2) * exp(-k (j-sy_s)^2)
            = sum_s U[s,i] * V[s,j]                    (rank-15 outer product)
    md2 ~= -ln(p) / k_b        (softmin; error ~ -ln(n_eff)/k, |.| <= 2.7 w^2/C)

so the whole distance field + min collapses into ONE K=30 bf16 matmul per
curve: lhsT[30,128] = U by (x-half, s), rhs[30,512] block-diagonal over
(x-half, y) with V blocks; out = one full PSUM bank [128, 512].  8 matmuls
fill all 8 banks = the whole per-core canvas [128, 4096] fp32.
C = 86 puts the fp32/bf16 exp underflow horizon (e^-87.3) right at the
clip boundary md = w: far pixels flush to p = 0 -> ln -> -inf -> canvas 0,
matching the reference's clipped zeros.  Host-side numpy sim of the full
dtype pipeline (bf16 U/V, fp32 PSUM w/ FTZ, bf16 tail) measures global
rel L2 error 3.7e-3 vs the fp64 reference (tolerance 2e-2); hardware run
matches.

The hardware Ln spline is only exact on [2^-64, 2^64] (saturates below,
garbage above) -- measured on device -- so Ln#1 prescales by 2^60 and the
offset is removed via clamp + Ln#2 bias (a const AP registered at init).

Tail over the full per-core canvas [128, 4096] (8 curves x 2 halves x 256):
    t  = Ln(p * 2^60)                   ACT, PSUM -> SBUF fp32
    t2 = clamp(t, LN_OFF-88.75, 41.56)  DVE one ts (min,max), fp32
    s2 = Ln(-t2/C + LN_OFF/C)           ACT -> bf16  ( = ln(md2/w^2) )
    u  = s2 * (aa/2)                    DVE tt, per-curve operand tile
    r  = Exp(u)                         ACT  ( = (md/w)^aa )
    o  = max(1 - r, 0)                  DVE ts x2 -> bf16 out
Ln+Exp share one ACT table set.  On the hardware cost model ACT is the
steady-state bottleneck: 3 passes x (4096+352)/1.2GHz ~= 11.1 us/iter
(CoreSim: 10.8 us/iter marginal, ~10x the 107.9 us baseline).  The
instruction count per iteration (8 LDW+MM pairs, 3 ACT, 4 DVE, 1 DMA) is
kept minimal because the bench runtime adds a large per-instruction cost.

Device decomposition: data-parallel over curves, core c owns curves
[8c, 8c+8); no cross-core communication.  Output bf16, cast on host.
"""

import numpy as np
from math import comb

H = W = 256
S = 15
B = 64
NCORES = 8
CPB = B // NCORES          # curves per core
UNITS = CPB * 2            # (curve, x-half) units per core

C_SOFT = 86.0              # softmin sharpness: k = C/w^2
FLUSH = 1.1755e-38         # fp32/bf16 min normal: pre-flush denormal U/V

# The hardware Ln spline is exact only on [2^-64, 2^64] (saturates below,
# garbage above).  p spans [2^-126, 2^4], so Ln#1 prescales by 2^61:
# t = Ln(p * 2^60) = ln p + 60 ln 2, keeping inputs in [2^-66.2, 2^64).
# The offset is removed by the clamp + Ln#2 bias.  z = LN_OFF - t.
# z is clamped to <= C_SOFT exactly, so md2/w^2 <= 1, r = (md2/w^2)^q <= 1,
# and o = 1 - r needs no final max(., 0): far pixels (p underflowed) hit
# z = C -> s2 = ln 1 = 0 -> r = 1 -> o = 0, matching the reference's clip.
LN_SCALE_E = 60
LN_OFF = LN_SCALE_E * float(np.log(2.0))      # 42.2804
T_CLAMP = 41.5625          # fp16-exact; z >= LN_OFF - T_CLAMP = 0.027 > 0
T_FLOOR = LN_OFF - C_SOFT  # -43.7196; t >= T_FLOOR <=> z <= C_SOFT

_prog_cache = {}


# ---------------------------------------------------------------------------
# host-side math
# ---------------------------------------------------------------------------

def _bezier_samples(inputs_np):
    """[B,S,2] float64 sample points (y, x) in pixel coords."""
    kp = inputs_np.astype(np.float64) * np.array([H, W], np.float64)
    K = kp.shape[1]
    ts = np.linspace(0.0, 1.0, S)
    k = np.arange(K)
    binom = np.array([comb(K - 1, i) for i in range(K)], np.float64)
    basis = binom * ts[:, None] ** k * (1.0 - ts[:, None]) ** (K - 1 - k)
    return np.einsum("sk,bkd->bsd", basis, kp)


def _make_core_inputs(sp, widths, aas, core):
    """Input tensors for one core (curves [8*core, 8*core+8))."""
    import ml_dtypes

    bf16 = ml_dtypes.bfloat16
    coords = np.arange(256, dtype=np.float64)
    # one K=30 matmul per curve: rows (h, s); lhsT[30, 128] = U split by
    # x-half, rhs[30, 512] block-diagonal over (h, y) with V in each block
    ut = np.zeros((2 * S, CPB * 128), np.float32)
    vt = np.zeros((2 * S, CPB * 512), np.float32)
    aq = np.zeros((128, CPB), np.float32)          # aa/2 per curve column
    with np.errstate(under="ignore"):
        for cl in range(CPB):
            b = core * CPB + cl
            kb = C_SOFT / float(widths[b]) ** 2
            sy, sx = sp[b, :, 0], sp[b, :, 1]
            U = np.exp(-kb * (coords[None, :] - sx[:, None]) ** 2)  # [S,256]
            V = np.exp(-kb * (coords[None, :] - sy[:, None]) ** 2)  # [S,256]
            for h in range(2):
                ut[h * S:(h + 1) * S, cl * 128:(cl + 1) * 128] = \
                    U[:, h * 128:(h + 1) * 128]
                vt[h * S:(h + 1) * S, cl * 512 + h * 256:cl * 512 + (h + 1) * 256] = V
            aq[:, cl] = np.float32(aas[b] / 2.0)
    ut[ut < FLUSH] = 0.0
    vt[vt < FLUSH] = 0.0
    utb = ut.astype(bf16)
    vtb = vt.astype(bf16)
    # flush any bf16 denormals produced by the rounding itself
    utb[utb.astype(np.float32) < FLUSH] = 0
    vtb[vtb.astype(np.float32) < FLUSH] = 0
    return {"ut": utb, "vt": vtb, "aq": aq}


# ---------------------------------------------------------------------------
# multi-wait workaround
# ---------------------------------------------------------------------------

def _split_multi_waits(nc):
    """This walrus build accepts only one sync-wait per instruction.  Hoist
    extra waits onto same-engine nops inserted just before the instruction
    (engine program order makes this semantically identical: all waits retire
    before the instruction issues)."""
    import concourse.mybir as mybir

    n = 0
    for fn in nc.m.functions:
        for bb in fn.blocks:
            insts = list(bb.instructions)
            out = []
            changed = False
            for inst in insts:
                si = inst.sync_info
                if si is not None and len(si.on_wait) > 1:
                    waits = list(si.on_wait)
                    for i, w in enumerate(waits[:-1]):
                        nop = mybir.InstNoOp(name=f"{inst.name}_xw{i}")
                        nop.engine = inst.engine
                        nop.sync_info = mybir.SyncInfo(on_wait=[w], on_update=[])
                        out.append(nop)
                        n += 1
                    inst.sync_info = mybir.SyncInfo(
                        on_wait=[waits[-1]], on_update=list(si.on_update)
                    )
                    changed = True
                out.append(inst)
            if changed:
                bb.instructions = out
    return n


# ---------------------------------------------------------------------------
# bass program (input-independent structure)
# ---------------------------------------------------------------------------

def _build_program(repeat=1, num_devices=NCORES):
    import concourse.bass as bass
    import concourse.mybir as mybir
    from concourse.tile import TileContext

    fp32 = mybir.dt.float32
    fp16 = mybir.dt.float16
    bf16 = mybir.dt.bfloat16
    A = mybir.AluOpType
    F = mybir.ActivationFunctionType

    nc = bass.Bass("TRN2", target_bir_lowering=False, debug=False,
                   num_devices=num_devices)
    bias_val = LN_OFF / C_SOFT
    _tb = nc.alloc_sbuf_tensor("const-float32-ln2bias", [128, 1], fp32)
    nc.gpsimd.memset(_tb.ap(), bias_val)
    nc.const_aps.aps[(fp32, bias_val)] = _tb.ap()
    nc.all_engine_barrier()
    ut_d = nc.dram_tensor("ut", [2 * S, CPB * 128], bf16, kind="ExternalInput")
    vt_d = nc.dram_tensor("vt", [2 * S, CPB * 512], bf16, kind="ExternalInput")
    aq_d = nc.dram_tensor("aq", [128, CPB], fp32, kind="ExternalInput")
    out_d = nc.dram_tensor("out", [128, UNITS * 256], bf16,
                           kind="ExternalOutput")

    with TileContext(nc) as tc:
        with (
            tc.tile_pool(name="const", bufs=1) as constp,
            tc.tile_pool(name="tail", bufs=2) as tailp,
            tc.tile_pool(name="ot", bufs=2) as otp,
            tc.psum_pool(name="psum", bufs=1) as psp,
        ):
            ut = constp.tile([2 * S, CPB * 128], bf16, tag="ut")
            nc.sync.dma_start(out=ut[:], in_=ut_d[:])
            vt = constp.tile([2 * S, CPB * 512], bf16, tag="vt")
            nc.sync.dma_start(out=vt[:], in_=vt_d[:])
            aq = constp.tile([128, CPB], fp32, tag="aq")
            nc.sync.dma_start(out=aq[:], in_=aq_d[:])

            pp = psp.tile([128, UNITS * 256], fp32, tag="pp")

            def body():
                t = tailp.tile([128, UNITS * 256], fp32, tag="t")
                t2 = tailp.tile([128, UNITS * 256], fp32, tag="t2")
                s2 = tailp.tile([128, UNITS * 256], bf16, tag="s2")
                uu = tailp.tile([128, UNITS * 256], bf16, tag="uu")
                rr = tailp.tile([128, UNITS * 256], bf16, tag="rr")
                oo = otp.tile([128, UNITS * 256], bf16, tag="oo")
                # p = sum_s U[s,i] V[s,j]; one K=30 matmul per curve = bank
                for cl in range(CPB):
                    nc.tensor.matmul(
                        pp[:, cl * 512:(cl + 1) * 512],
                        ut[:, cl * 128:(cl + 1) * 128],
                        vt[:, cl * 512:(cl + 1) * 512],
                        start=True, stop=True,
                    )
                # t = ln p + LN_OFF   (fp32)
                nc.scalar.activation(t[:], pp[:], F.Ln,
                                     scale=float(2.0 ** LN_SCALE_E))
                # t2 = clamp(t): z = LN_OFF - t2 in [0.027, C] => md2 <= w^2
                nc.vector.tensor_scalar(t2[:], t[:], T_CLAMP,
                                        T_FLOOR, A.min, A.max)
                # s2 = ln((LN_OFF - t2)/C) = ln(z/C) = ln(md2/w^2)
                nc.scalar.activation(s2[:], t2[:], F.Ln, scale=-1.0 / C_SOFT,
                                     bias=bias_val)
                # u = (aa/2) ln(md2/w^2); aa/2 broadcast per curve block
                for cl in range(CPB):
                    nc.vector.tensor_scalar_mul(
                        out=uu[:, cl * 512:(cl + 1) * 512],
                        in0=s2[:, cl * 512:(cl + 1) * 512],
                        scalar1=aq[:, cl:cl + 1],
                    )
                # r = (md/w)^aa
                nc.scalar.activation(rr[:], uu[:], F.Exp)
                # o = 1 - r  (r <= 1 by the z clamp: no final max needed)
                nc.vector.tensor_scalar(oo[:], rr[:], -1.0, 1.0, A.mult, A.add)
                nc.sync.dma_start(out=out_d[:], in_=oo[:])

            for _ in range(repeat):
                body()
    _split_multi_waits(nc)
    return nc


# ---------------------------------------------------------------------------
# public entry point
# ---------------------------------------------------------------------------

def _run(inputs, widths, aa_factors, repeat=1):
    from concourse.bass_utils import run_bass_kernel_spmd

    inputs = np.asarray(inputs, np.float32)
    widths = np.asarray(widths, np.float32)
    aa_factors = np.asarray(aa_factors, np.float32)
    assert inputs.shape == (B, 4, 2), inputs.shape

    sp = _bezier_samples(inputs)
    if repeat not in _prog_cache:
        _prog_cache[repeat] = _build_program(repeat)
    nc = _prog_cache[repeat]

    in_maps = [
        _make_core_inputs(sp, widths, aa_factors, c) for c in range(NCORES)
    ]
    res = run_bass_kernel_spmd(nc, in_maps, list(range(NCORES)))

    canvas = np.empty((B, H, W), np.float32)
    for c in range(NCORES):
        out = np.asarray(res.results[c]["out"])          # [128, 4096] fp16
        out = out.reshape(128, CPB, 2, 256)              # [i, cl, h, y]
        out = out.transpose(1, 2, 0, 3)                  # [cl, h, i, y]
        canvas[c * CPB:(c + 1) * CPB] = out.reshape(CPB, 256, 256)
    return canvas


def kernel(inputs, widths, aa_factors):
    return _run(inputs, widths, aa_factors, repeat=1)



# revision 10
# speedup vs baseline: 39.8932x; 1.3393x over previous
"""Bass/Trainium2 kernel for nn_CurveGraphic2d (min-distance curve rasterizer).

kernel(**inputs) takes FULL inputs (inputs [64,4,2] f32, widths [64] f32,
aa_factors [64] f32) and returns the FULL [64,256,256] float32 canvas.

Math (per curve b, output element [b, i, j]; reference flattens its pixel
grid x-major, so row index i is the x coordinate, column j is y):

    md2    = min_s (j - sy_bs)^2 + (i - sx_bs)^2
    canvas = clip(1 - (md2/w_b^2)^(aa_b/2), 0, 1)

Softmin-by-matmul: with k_b = C / w_b^2,

    p(i,j)  = sum_s exp(-k d2_s) = sum_s exp(-k (i-sx_s)^2) * exp(-k (j-sy_s)^2)
            = sum_s U[s,i] * V[s,j]                    (rank-15 outer product)
    md2 ~= -ln(p) / k_b        (softmin; error ~ -ln(n_eff)/k, |.| <= 2.7 w^2/C)

so the whole distance field + min collapses into ONE K=30 bf16 matmul per
curve: lhsT[30,128] = U by (x-half, s), rhs[30,512] block-diagonal over
(x-half, y) with V blocks; out = one full PSUM bank [128, 512].  8 matmuls
fill all 8 banks = the whole per-core canvas [128, 4096] fp32.
C = 86 puts the fp32/bf16 exp underflow horizon (e^-87.3) right at the
clip boundary md = w: far pixels flush to p = 0 -> ln -> -inf -> canvas 0,
matching the reference's clipped zeros.  Host-side numpy sim of the full
dtype pipeline (bf16 U/V, fp32 PSUM w/ FTZ, bf16 tail) measures global
rel L2 error 3.7e-3 vs the fp64 reference (tolerance 2e-2); hardware run
matches.

The hardware Ln spline is only exact on [2^-64, 2^64] (saturates below,
garbage above) -- measured on device -- so Ln#1 prescales by 2^60 and the
offset is removed via clamp + Ln#2 bias (a const AP registered at init).

Tail over the full per-core canvas [128, 4096] (8 curves x 2 halves x 256):
    t  = Ln(p * 2^60)                   ACT, PSUM -> SBUF fp32
    t2 = clamp(t, LN_OFF-88.75, 41.56)  DVE one ts (min,max), fp32
    s2 = Ln(-t2/C + LN_OFF/C)           ACT -> bf16  ( = ln(md2/w^2) )
    u  = s2 * (aa/2)                    DVE tt, per-curve operand tile
    r  = Exp(u)                         ACT  ( = (md/w)^aa )
    o  = max(1 - r, 0)                  DVE ts x2 -> bf16 out
Ln+Exp share one ACT table set.  On the hardware cost model ACT is the
steady-state bottleneck: 3 passes x (4096+352)/1.2GHz ~= 11.1 us/iter
(CoreSim: 10.8 us/iter marginal, ~10x the 107.9 us baseline).  The
instruction count per iteration (8 LDW+MM pairs, 3 ACT, 4 DVE, 1 DMA) is
kept minimal because the bench runtime adds a large per-instruction cost.

Device decomposition: data-parallel over curves, core c owns curves
[8c, 8c+8); no cross-core communication.  Output bf16, cast on host.
"""

import numpy as np
from math import comb

H = W = 256
S = 15
B = 64
NCORES = 8
CPB = B // NCORES          # curves per core
UNITS = CPB * 2            # (curve, x-half) units per core

C_SOFT = 86.0              # softmin sharpness: k = C/w^2
FLUSH = 1.1755e-38         # fp32/bf16 min normal: pre-flush denormal U/V

# The hardware Ln spline is exact only on [2^-64, 2^64] (saturates below,
# garbage above).  p spans [2^-126, 2^4], so Ln#1 prescales by 2^61:
# t = Ln(p * 2^60) = ln p + 60 ln 2, keeping inputs in [2^-66.2, 2^64).
# The offset is removed by the clamp + Ln#2 bias.  z = LN_OFF - t.
# z is clamped to <= C_SOFT exactly, so md2/w^2 <= 1, r = (md2/w^2)^q <= 1,
# and o = 1 - r needs no final max(., 0): far pixels (p underflowed) hit
# z = C -> s2 = ln 1 = 0 -> r = 1 -> o = 0, matching the reference's clip.
LN_SCALE_E = 60
LN_OFF = LN_SCALE_E * float(np.log(2.0))      # 42.2804
T_CLAMP = 41.5625          # fp16-exact; z >= LN_OFF - T_CLAMP = 0.027 > 0
T_FLOOR = LN_OFF - C_SOFT  # -43.7196; t >= T_FLOOR <=> z <= C_SOFT

_prog_cache = {}


# ---------------------------------------------------------------------------
# host-side math
# ---------------------------------------------------------------------------

def _bezier_samples(inputs_np):
    """[B,S,2] float64 sample points (y, x) in pixel coords."""
    kp = inputs_np.astype(np.float64) * np.array([H, W], np.float64)
    K = kp.shape[1]
    ts = np.linspace(0.0, 1.0, S)
    k = np.arange(K)
    binom = np.array([comb(K - 1, i) for i in range(K)], np.float64)
    basis = binom * ts[:, None] ** k * (1.0 - ts[:, None]) ** (K - 1 - k)
    return np.einsum("sk,bkd->bsd", basis, kp)


def _make_core_inputs(sp, widths, aas, core):
    """Input tensors for one core (curves [8*core, 8*core+8))."""
    import ml_dtypes

    bf16 = ml_dtypes.bfloat16
    coords = np.arange(256, dtype=np.float64)
    # one K=30 matmul per curve: rows (h, s); lhsT[30, 128] = U split by
    # x-half, rhs[30, 512] block-diagonal over (h, y) with V in each block
    ut = np.zeros((2 * S, CPB * 128), np.float32)
    vt = np.zeros((2 * S, CPB * 512), np.float32)
    at = np.zeros((128, UNITS * 256), np.float32)  # aa/2 per curve block
    with np.errstate(under="ignore"):
        for cl in range(CPB):
            b = core * CPB + cl
            kb = C_SOFT / float(widths[b]) ** 2
            sy, sx = sp[b, :, 0], sp[b, :, 1]
            U = np.exp(-kb * (coords[None, :] - sx[:, None]) ** 2)  # [S,256]
            V = np.exp(-kb * (coords[None, :] - sy[:, None]) ** 2)  # [S,256]
            for h in range(2):
                ut[h * S:(h + 1) * S, cl * 128:(cl + 1) * 128] = \
                    U[:, h * 128:(h + 1) * 128]
                vt[h * S:(h + 1) * S, cl * 512 + h * 256:cl * 512 + (h + 1) * 256] = V
            at[:, cl * 512:(cl + 1) * 512] = np.float32(aas[b] / 2.0)
    ut[ut < FLUSH] = 0.0
    vt[vt < FLUSH] = 0.0
    utb = ut.astype(bf16)
    vtb = vt.astype(bf16)
    # flush any bf16 denormals produced by the rounding itself
    utb[utb.astype(np.float32) < FLUSH] = 0
    vtb[vtb.astype(np.float32) < FLUSH] = 0
    bias = np.full((128, 1), LN_OFF / C_SOFT, np.float32)
    return {"ut": utb, "vt": vtb, "at": at.astype(bf16), "bias": bias}


# ---------------------------------------------------------------------------
# multi-wait workaround
# ---------------------------------------------------------------------------

def _split_multi_waits(nc):
    """This walrus build accepts only one sync-wait per instruction.  Hoist
    extra waits onto same-engine nops inserted just before the instruction
    (engine program order makes this semantically identical: all waits retire
    before the instruction issues)."""
    import concourse.mybir as mybir

    n = 0
    for fn in nc.m.functions:
        for bb in fn.blocks:
            insts = list(bb.instructions)
            out = []
            changed = False
            for inst in insts:
                si = inst.sync_info
                if si is not None and len(si.on_wait) > 1:
                    waits = list(si.on_wait)
                    for i, w in enumerate(waits[:-1]):
                        nop = mybir.InstNoOp(name=f"{inst.name}_xw{i}")
                        nop.engine = inst.engine
                        nop.sync_info = mybir.SyncInfo(on_wait=[w], on_update=[])
                        out.append(nop)
                        n += 1
                    inst.sync_info = mybir.SyncInfo(
                        on_wait=[waits[-1]], on_update=list(si.on_update)
                    )
                    changed = True
                out.append(inst)
            if changed:
                bb.instructions = out
    return n


# ---------------------------------------------------------------------------
# bass program (input-independent structure)
# ---------------------------------------------------------------------------

def _build_program(repeat=1, num_devices=NCORES):
    import concourse.bass as bass
    import concourse.mybir as mybir
    from concourse.tile import TileContext

    fp32 = mybir.dt.float32
    fp16 = mybir.dt.float16
    bf16 = mybir.dt.bfloat16
    A = mybir.AluOpType
    F = mybir.ActivationFunctionType

    nc = bass.Bass("TRN2", target_bir_lowering=False, debug=False,
                   num_devices=num_devices)
    ut_d = nc.dram_tensor("ut", [2 * S, CPB * 128], bf16, kind="ExternalInput")
    vt_d = nc.dram_tensor("vt", [2 * S, CPB * 512], bf16, kind="ExternalInput")
    at_d = nc.dram_tensor("at", [128, UNITS * 256], bf16, kind="ExternalInput")
    bias_d = nc.dram_tensor("bias", [128, 1], fp32, kind="ExternalInput")
    out_d = nc.dram_tensor("out", [128, UNITS * 256], bf16,
                           kind="ExternalOutput")

    HCURVE = CPB // 2          # curves per half-canvas chunk
    HCOL = HCURVE * 512        # 2048 columns per chunk

    with TileContext(nc) as tc:
        with (
            tc.tile_pool(name="const", bufs=1) as constp,
            tc.tile_pool(name="tail", bufs=2) as tailp,
            tc.tile_pool(name="ot", bufs=2) as otp,
            tc.psum_pool(name="psum", bufs=1) as psp,
        ):
            ut = constp.tile([2 * S, CPB * 128], bf16, tag="ut")
            vt = constp.tile([2 * S, CPB * 512], bf16, tag="vt")
            # split by half so the first matmuls start after half the bytes
            for h in range(2):
                nc.sync.dma_start(
                    out=ut[:, h * HCURVE * 128:(h + 1) * HCURVE * 128],
                    in_=ut_d[:, h * HCURVE * 128:(h + 1) * HCURVE * 128])
                nc.sync.dma_start(
                    out=vt[:, h * HCOL:(h + 1) * HCOL],
                    in_=vt_d[:, h * HCOL:(h + 1) * HCOL])
            bias = constp.tile([128, 1], fp32, tag="bias")
            nc.sync.dma_start(out=bias[:], in_=bias_d[:])
            at = constp.tile([128, UNITS * 256], bf16, tag="at")
            nc.sync.dma_start(out=at[:], in_=at_d[:])

            pp = psp.tile([128, UNITS * 256], fp32, tag="pp")

            def body():
                t = tailp.tile([128, UNITS * 256], fp32, tag="t")
                t2 = tailp.tile([128, UNITS * 256], fp32, tag="t2")
                s2 = tailp.tile([128, UNITS * 256], bf16, tag="s2")
                uu = tailp.tile([128, UNITS * 256], bf16, tag="uu")
                rr = tailp.tile([128, UNITS * 256], bf16, tag="rr")
                oo = otp.tile([128, UNITS * 256], bf16, tag="oo")

                def cols(h):
                    return slice(h * HCOL, (h + 1) * HCOL)

                def mm(h):
                    # p = sum_s U[s,i] V[s,j]; one K=30 matmul per curve/bank
                    for cl in range(h * HCURVE, (h + 1) * HCURVE):
                        nc.tensor.matmul(
                            pp[:, cl * 512:(cl + 1) * 512],
                            ut[:, cl * 128:(cl + 1) * 128],
                            vt[:, cl * 512:(cl + 1) * 512],
                            start=True, stop=True,
                        )

                def ln1(h):
                    # t = ln p + LN_OFF   (fp32)
                    nc.scalar.activation(t[:, cols(h)], pp[:, cols(h)], F.Ln,
                                         scale=float(2.0 ** LN_SCALE_E))

                def clamp(h):
                    # z = LN_OFF - t2 in [0.027, C] => md2 <= w^2, r <= 1
                    nc.vector.tensor_scalar(t2[:, cols(h)], t[:, cols(h)],
                                            T_CLAMP, T_FLOOR, A.min, A.max)

                def ln2(h):
                    # s2 = ln((LN_OFF - t2)/C) = ln(md2/w^2)
                    nc.scalar.activation(s2[:, cols(h)], t2[:, cols(h)], F.Ln,
                                         scale=-1.0 / C_SOFT, bias=bias[:])

                def mul(h):
                    # u = (aa/2) ln(md2/w^2)
                    nc.vector.tensor_tensor(uu[:, cols(h)], s2[:, cols(h)],
                                            at[:, cols(h)], A.mult)

                def expf(h):
                    # r = (md/w)^aa
                    nc.scalar.activation(rr[:, cols(h)], uu[:, cols(h)], F.Exp)

                def fin(h):
                    # o = 1 - r  (r <= 1 by the z clamp: no final max needed)
                    nc.vector.tensor_scalar(oo[:, cols(h)], rr[:, cols(h)],
                                            -1.0, 1.0, A.mult, A.add)
                    nc.sync.dma_start(out=out_d[:, cols(h)],
                                      in_=oo[:, cols(h)])

                # per-engine queue order = emission order; interleaving the
                # halves one stage apart lets ACT(h1) run under DVE(h0) etc.
                for op in (mm, ln1, clamp, ln2, mul, expf, fin):
                    op(0)
                    op(1)

            for _ in range(repeat):
                body()
    _split_multi_waits(nc)
    return nc


# ---------------------------------------------------------------------------
# public entry point
# ---------------------------------------------------------------------------

def _run(inputs, widths, aa_factors, repeat=1):
    from concourse.bass_utils import run_bass_kernel_spmd

    inputs = np.asarray(inputs, np.float32)
    widths = np.asarray(widths, np.float32)
    aa_factors = np.asarray(aa_factors, np.float32)
    assert inputs.shape == (B, 4, 2), inputs.shape

    sp = _bezier_samples(inputs)
    if repeat not in _prog_cache:
        _prog_cache[repeat] = _build_program(repeat)
    nc = _prog_cache[repeat]

    in_maps = [
        _make_core_inputs(sp, widths, aa_factors, c) for c in range(NCORES)
    ]
    res = run_bass_kernel_spmd(nc, in_maps, list(range(NCORES)))

    canvas = np.empty((B, H, W), np.float32)
    for c in range(NCORES):
        out = np.asarray(res.results[c]["out"])          # [128, 4096] fp16
        out = out.reshape(128, CPB, 2, 256)              # [i, cl, h, y]
        out = out.transpose(1, 2, 0, 3)                  # [cl, h, i, y]
        canvas[c * CPB:(c + 1) * CPB] = out.reshape(CPB, 256, 256)
    return canvas


def kernel(inputs, widths, aa_factors):
    return _run(inputs, widths, aa_factors, repeat=1)



# revision 12
# speedup vs baseline: 110.0335x; 2.7582x over previous
"""Bass/Trainium2 kernel for nn_CurveGraphic2d (min-distance curve rasterizer).

kernel(**inputs) takes FULL inputs (inputs [64,4,2] f32, widths [64] f32,
aa_factors [64] f32) and returns the FULL [64,256,256] float32 canvas.

Math (per curve b, output element [b, i, j]; reference flattens its pixel
grid x-major, so row index i is the x coordinate, column j is y):

    md2    = min_s (j - sy_bs)^2 + (i - sx_bs)^2
    canvas = clip(1 - (md2/w_b^2)^(aa_b/2), 0, 1)

Softmin-by-matmul: with k_b = C / w_b^2,

    p(i,j)  = sum_s exp(-k d2_s) = sum_s exp(-k (i-sx_s)^2) * exp(-k (j-sy_s)^2)
            = sum_s U[s,i] * V[s,j]                    (rank-15 outer product)
    md2 ~= -ln(p) / k_b        (softmin; error ~ -ln(n_eff)/k, |.| <= 2.7 w^2/C)

so the whole distance field + min collapses into ONE K=30 bf16 matmul per
curve: lhsT[30,128] = U by (x-half, s), rhs[30,512] block-diagonal over
(x-half, y) with V blocks; out = one full PSUM bank [128, 512].  8 matmuls
fill all 8 banks = the whole per-core canvas [128, 4096] fp32.
C = 86 puts the fp32/bf16 exp underflow horizon (e^-87.3) right at the
clip boundary md = w: far pixels flush to p = 0 -> ln -> -inf -> canvas 0,
matching the reference's clipped zeros.  Host-side numpy sim of the full
dtype pipeline (bf16 U/V, fp32 PSUM w/ FTZ, bf16 tail) measures global
rel L2 error 3.7e-3 vs the fp64 reference (tolerance 2e-2); hardware run
matches.

The hardware Ln spline is only exact on [2^-64, 2^64] (saturates below,
garbage above) -- measured on device -- so Ln#1 prescales by 2^60 and the
offset is removed via clamp + Ln#2 bias (a [128,1] fp32 input tensor).

Tail over the full per-core canvas [128, 4096] (8 curves x 2 halves x 256):
    t  = Ln(p * 2^60)                   ACT, PSUM -> SBUF fp32
    t2 = clamp(t, LN_OFF-C, 41.56)      DVE one ts (min,max), fp32
    s2 = Ln(-t2/C + LN_OFF/C)           ACT -> bf16  ( = ln(md2/w^2) )
    u  = s2 * (aa/2)                    DVE tt, per-curve operand tile
    r  = Exp(u)                         ACT  ( = (md/w)^aa )
    o  = 1 - r                          DVE one ts -> bf16 out
The z-clamp upper bound is exactly C, so md2/w^2 <= 1, r <= 1, and no
final max(, 0) is needed: underflowed far pixels land exactly on o = 0.
Ln+Exp share one ACT table set.  ACT is the steady-state bottleneck and is
measured 100% busy: marginal cost/iteration = 3 passes x 4096/1.2GHz +
3 x 167ns effective instruction overhead ~= 10.75 us (vs the 2.9 us/core
HBM write roofline; headroom 8 vs the 107.9 us jax baseline is met ~10x
over).  Full-width (un-chunked) tail passes minimize per-instruction
overhead on the saturated engine; tail tiles are double-buffered so
iteration i+1's ACT overlaps iteration i's DVE.

Device decomposition: data-parallel over curves, core c owns curves
[8c, 8c+8); no cross-core communication.  Output bf16, cast on host.
"""

import numpy as np
from math import comb

H = W = 256
S = 15
B = 64
NCORES = 8
CPB = B // NCORES          # curves per core
UNITS = CPB * 2            # (curve, x-half) units per core

C_SOFT = 86.0              # softmin sharpness: k = C/w^2
FLUSH = 1.1755e-38         # fp32/bf16 min normal: pre-flush denormal U/V

# The hardware Ln spline is exact only on [2^-64, 2^64] (saturates below,
# garbage above).  p spans [2^-126, 2^4], so Ln#1 prescales by 2^61:
# t = Ln(p * 2^60) = ln p + 60 ln 2, keeping inputs in [2^-66.2, 2^64).
# The offset is removed by the clamp + Ln#2 bias.  z = LN_OFF - t.
# z is clamped to <= C_SOFT exactly, so md2/w^2 <= 1, r = (md2/w^2)^q <= 1,
# and o = 1 - r needs no final max(., 0): far pixels (p underflowed) hit
# z = C -> s2 = ln 1 = 0 -> r = 1 -> o = 0, matching the reference's clip.
LN_SCALE_E = 60
LN_OFF = LN_SCALE_E * float(np.log(2.0))      # 42.2804
T_CLAMP = 41.5625          # fp16-exact; z >= LN_OFF - T_CLAMP = 0.027 > 0
T_FLOOR = LN_OFF - C_SOFT  # -43.7196; t >= T_FLOOR <=> z <= C_SOFT

_prog_cache = {}


# ---------------------------------------------------------------------------
# host-side math
# ---------------------------------------------------------------------------

def _bezier_samples(inputs_np):
    """[B,S,2] float64 sample points (y, x) in pixel coords."""
    kp = inputs_np.astype(np.float64) * np.array([H, W], np.float64)
    K = kp.shape[1]
    ts = np.linspace(0.0, 1.0, S)
    k = np.arange(K)
    binom = np.array([comb(K - 1, i) for i in range(K)], np.float64)
    basis = binom * ts[:, None] ** k * (1.0 - ts[:, None]) ** (K - 1 - k)
    return np.einsum("sk,bkd->bsd", basis, kp)


def _make_core_inputs(sp, widths, aas, core):
    """Input tensors for one core (curves [8*core, 8*core+8))."""
    import ml_dtypes

    bf16 = ml_dtypes.bfloat16
    coords = np.arange(256, dtype=np.float64)
    # one K=30 matmul per curve: rows (h, s); lhsT[30, 128] = U split by
    # x-half, rhs[30, 512] block-diagonal over (h, y) with V in each block
    ut = np.zeros((2 * S, CPB * 128), np.float32)
    vt = np.zeros((2 * S, CPB * 512), np.float32)
    at = np.zeros((128, UNITS * 256), np.float32)  # aa/2 per curve block
    with np.errstate(under="ignore"):
        for cl in range(CPB):
            b = core * CPB + cl
            kb = C_SOFT / float(widths[b]) ** 2
            sy, sx = sp[b, :, 0], sp[b, :, 1]
            U = np.exp(-kb * (coords[None, :] - sx[:, None]) ** 2)  # [S,256]
            V = np.exp(-kb * (coords[None, :] - sy[:, None]) ** 2)  # [S,256]
            for h in range(2):
                ut[h * S:(h + 1) * S, cl * 128:(cl + 1) * 128] = \
                    U[:, h * 128:(h + 1) * 128]
                vt[h * S:(h + 1) * S, cl * 512 + h * 256:cl * 512 + (h + 1) * 256] = V
            at[:, cl * 512:(cl + 1) * 512] = np.float32(aas[b] / 2.0)
    ut[ut < FLUSH] = 0.0
    vt[vt < FLUSH] = 0.0
    utb = ut.astype(bf16)
    vtb = vt.astype(bf16)
    # flush any bf16 denormals produced by the rounding itself
    utb[utb.astype(np.float32) < FLUSH] = 0
    vtb[vtb.astype(np.float32) < FLUSH] = 0
    bias = np.full((128, 1), LN_OFF / C_SOFT, np.float32)
    return {"ut": utb, "vt": vtb, "at": at.astype(bf16), "bias": bias}


# ---------------------------------------------------------------------------
# multi-wait workaround
# ---------------------------------------------------------------------------

def _split_multi_waits(nc):
    """This walrus build accepts only one sync-wait per instruction.  Hoist
    extra waits onto same-engine nops inserted just before the instruction
    (engine program order makes this semantically identical: all waits retire
    before the instruction issues)."""
    import concourse.mybir as mybir

    n = 0
    for fn in nc.m.functions:
        for bb in fn.blocks:
            insts = list(bb.instructions)
            out = []
            changed = False
            for inst in insts:
                si = inst.sync_info
                if si is not None and len(si.on_wait) > 1:
                    waits = list(si.on_wait)
                    for i, w in enumerate(waits[:-1]):
                        nop = mybir.InstNoOp(name=f"{inst.name}_xw{i}")
                        nop.engine = inst.engine
                        nop.sync_info = mybir.SyncInfo(on_wait=[w], on_update=[])
                        out.append(nop)
                        n += 1
                    inst.sync_info = mybir.SyncInfo(
                        on_wait=[waits[-1]], on_update=list(si.on_update)
                    )
                    changed = True
                out.append(inst)
            if changed:
                bb.instructions = out
    return n


# ---------------------------------------------------------------------------
# bass program (input-independent structure)
# ---------------------------------------------------------------------------

def _build_program(repeat=1, num_devices=NCORES):
    import concourse.bass as bass
    import concourse.mybir as mybir
    from concourse.tile import TileContext

    fp32 = mybir.dt.float32
    fp16 = mybir.dt.float16
    bf16 = mybir.dt.bfloat16
    A = mybir.AluOpType
    F = mybir.ActivationFunctionType

    nc = bass.Bass("TRN2", target_bir_lowering=False, debug=False,
                   num_devices=num_devices)
    ut_d = nc.dram_tensor("ut", [2 * S, CPB * 128], bf16, kind="ExternalInput")
    vt_d = nc.dram_tensor("vt", [2 * S, CPB * 512], bf16, kind="ExternalInput")
    at_d = nc.dram_tensor("at", [128, UNITS * 256], bf16, kind="ExternalInput")
    bias_d = nc.dram_tensor("bias", [128, 1], fp32, kind="ExternalInput")
    out_d = nc.dram_tensor("out", [128, UNITS * 256], bf16,
                           kind="ExternalOutput")

    HCURVE = CPB // 2          # curves per half-canvas chunk
    HCOL = HCURVE * 512        # 2048 columns per chunk

    with TileContext(nc) as tc:
        with (
            tc.tile_pool(name="const", bufs=1) as constp,
            tc.tile_pool(name="tail", bufs=2) as tailp,
            tc.tile_pool(name="ot", bufs=2) as otp,
            tc.psum_pool(name="psum", bufs=1) as psp,
        ):
            ut = constp.tile([2 * S, CPB * 128], bf16, tag="ut")
            vt = constp.tile([2 * S, CPB * 512], bf16, tag="vt")
            # split by half so the first matmuls start after half the bytes
            for h in range(2):
                nc.sync.dma_start(
                    out=ut[:, h * HCURVE * 128:(h + 1) * HCURVE * 128],
                    in_=ut_d[:, h * HCURVE * 128:(h + 1) * HCURVE * 128])
                nc.sync.dma_start(
                    out=vt[:, h * HCOL:(h + 1) * HCOL],
                    in_=vt_d[:, h * HCOL:(h + 1) * HCOL])
            bias = constp.tile([128, 1], fp32, tag="bias")
            nc.sync.dma_start(out=bias[:], in_=bias_d[:])
            at = constp.tile([128, UNITS * 256], bf16, tag="at")
            nc.sync.dma_start(out=at[:], in_=at_d[:])

            pp = psp.tile([128, UNITS * 256], fp32, tag="pp")

            def body():
                t = tailp.tile([128, UNITS * 256], fp32, tag="t")
                t2 = tailp.tile([128, UNITS * 256], fp32, tag="t2")
                s2 = tailp.tile([128, UNITS * 256], bf16, tag="s2")
                uu = tailp.tile([128, UNITS * 256], bf16, tag="uu")
                rr = tailp.tile([128, UNITS * 256], bf16, tag="rr")
                oo = otp.tile([128, UNITS * 256], bf16, tag="oo")

                # p = sum_s U[s,i] V[s,j]; one K=30 matmul per curve = bank
                for cl in range(CPB):
                    nc.tensor.matmul(
                        pp[:, cl * 512:(cl + 1) * 512],
                        ut[:, cl * 128:(cl + 1) * 128],
                        vt[:, cl * 512:(cl + 1) * 512],
                        start=True, stop=True,
                    )
                # t = ln p + LN_OFF   (fp32)
                nc.scalar.activation(t[:], pp[:], F.Ln,
                                     scale=float(2.0 ** LN_SCALE_E))
                # t2 = clamp(t): z = LN_OFF - t2 in [0.027, C] => md2 <= w^2
                nc.vector.tensor_scalar(t2[:], t[:], T_CLAMP,
                                        T_FLOOR, A.min, A.max)
                # s2 = ln((LN_OFF - t2)/C) = ln(z/C) = ln(md2/w^2)
                nc.scalar.activation(s2[:], t2[:], F.Ln,
                                     scale=-1.0 / C_SOFT, bias=bias[:])
                # u = (aa/2) ln(md2/w^2)
                nc.vector.tensor_tensor(uu[:], s2[:], at[:], A.mult)
                # r = (md/w)^aa
                nc.scalar.activation(rr[:], uu[:], F.Exp)
                # o = 1 - r  (r <= 1 by the z clamp: no final max needed)
                nc.vector.tensor_scalar(oo[:], rr[:], -1.0, 1.0, A.mult, A.add)
                nc.sync.dma_start(out=out_d[:], in_=oo[:])

            for _ in range(repeat):
                body()
    _split_multi_waits(nc)
    return nc


# ---------------------------------------------------------------------------
# public entry point
# ---------------------------------------------------------------------------

def _run(inputs, widths, aa_factors, repeat=1):
    from concourse.bass_utils import run_bass_kernel_spmd

    inputs = np.asarray(inputs, np.float32)
    widths = np.asarray(widths, np.float32)
    aa_factors = np.asarray(aa_factors, np.float32)
    assert inputs.shape == (B, 4, 2), inputs.shape

    sp = _bezier_samples(inputs)
    if repeat not in _prog_cache:
        _prog_cache[repeat] = _build_program(repeat)
    nc = _prog_cache[repeat]

    in_maps = [
        _make_core_inputs(sp, widths, aa_factors, c) for c in range(NCORES)
    ]
    res = run_bass_kernel_spmd(nc, in_maps, list(range(NCORES)))

    canvas = np.empty((B, H, W), np.float32)
    for c in range(NCORES):
        out = np.asarray(res.results[c]["out"])          # [128, 4096] fp16
        out = out.reshape(128, CPB, 2, 256)              # [i, cl, h, y]
        out = out.transpose(1, 2, 0, 3)                  # [cl, h, i, y]
        canvas[c * CPB:(c + 1) * CPB] = out.reshape(CPB, 256, 256)
    return canvas


def kernel(inputs, widths, aa_factors):
    return _run(inputs, widths, aa_factors, repeat=1)



# revision 13
# speedup vs baseline: 110.3284x; 1.0027x over previous
"""Bass/Trainium2 kernel for nn_CurveGraphic2d (min-distance curve rasterizer).

kernel(**inputs) takes FULL inputs (inputs [64,4,2] f32, widths [64] f32,
aa_factors [64] f32) and returns the FULL [64,256,256] float32 canvas.

Math (per curve b, output element [b, i, j]; reference flattens its pixel
grid x-major, so row index i is the x coordinate, column j is y):

    md2    = min_s (j - sy_bs)^2 + (i - sx_bs)^2
    canvas = clip(1 - (md2/w_b^2)^(aa_b/2), 0, 1)

Softmin-by-matmul: with k_b = C / w_b^2,

    p(i,j)  = sum_s exp(-k d2_s) = sum_s exp(-k (i-sx_s)^2) * exp(-k (j-sy_s)^2)
            = sum_s U[s,i] * V[s,j]                    (rank-15 outer product)
    md2 ~= -ln(p) / k_b        (softmin; error ~ -ln(n_eff)/k, |.| <= 2.7 w^2/C)

so the whole distance field + min collapses into ONE K=30 bf16 matmul per
curve: lhsT[30,128] = U by (x-half, s), rhs[30,512] block-diagonal over
(x-half, y) with V blocks; out = one full PSUM bank [128, 512].  8 matmuls
fill all 8 banks = the whole per-core canvas [128, 4096] fp32.
C = 86 puts the fp32/bf16 exp underflow horizon (e^-87.3) right at the
clip boundary md = w: far pixels flush to p = 0 -> ln -> -inf -> canvas 0,
matching the reference's clipped zeros.  Host-side numpy sim of the full
dtype pipeline (bf16 U/V, fp32 PSUM w/ FTZ, bf16 tail) measures global
rel L2 error 3.7e-3 vs the fp64 reference (tolerance 2e-2); hardware run
matches.

The hardware Ln spline is only exact on [2^-64, 2^64] (saturates below,
garbage above) -- measured on device -- so Ln#1 prescales by 2^60 and the
offset is removed via clamp + Ln#2 bias (a [128,1] fp32 input tensor).

Tail over the full per-core canvas [128, 4096] (8 curves x 2 halves x 256):
    t  = Ln(p * 2^60)                   ACT, PSUM -> SBUF fp32
    t2 = clamp(t, LN_OFF-C, 41.56)      DVE one ts (min,max), fp32
    s2 = Ln(-t2/C + LN_OFF/C)           ACT -> bf16  ( = ln(md2/w^2) )
    u  = s2 * (aa/2)                    DVE tt, per-curve operand tile
    r  = Exp(u)                         ACT  ( = (md/w)^aa )
    o  = 1 - r                          DVE one ts -> bf16 out
The z-clamp upper bound is exactly C, so md2/w^2 <= 1, r <= 1, and no
final max(, 0) is needed: underflowed far pixels land exactly on o = 0.
Ln+Exp share one ACT table set.  ACT is the steady-state bottleneck and is
measured 100% busy: marginal cost/iteration = 3 passes x 4096/1.2GHz +
3 x 167ns effective instruction overhead ~= 10.75 us (vs the 2.9 us/core
HBM write roofline; headroom 8 vs the 107.9 us jax baseline is met ~10x
over).  Full-width (un-chunked) tail passes minimize per-instruction
overhead on the saturated engine; tail tiles are double-buffered so
iteration i+1's ACT overlaps iteration i's DVE.

Device decomposition: data-parallel over curves, core c owns curves
[8c, 8c+8); no cross-core communication.  Output bf16, cast on host.
"""

import numpy as np
from math import comb

H = W = 256
S = 15
B = 64
NCORES = 8
CPB = B // NCORES          # curves per core
UNITS = CPB * 2            # (curve, x-half) units per core

C_SOFT = 86.0              # softmin sharpness: k = C/w^2
FLUSH = 1.1755e-38         # fp32/bf16 min normal: pre-flush denormal U/V

# The hardware Ln spline is exact only on [2^-64, 2^64] (saturates below,
# garbage above).  p spans [2^-126, 2^4], so Ln#1 prescales by 2^61:
# t = Ln(p * 2^60) = ln p + 60 ln 2, keeping inputs in [2^-66.2, 2^64).
# The offset is removed by the clamp + Ln#2 bias.  z = LN_OFF - t.
# z is clamped to <= C_SOFT exactly, so md2/w^2 <= 1, r = (md2/w^2)^q <= 1,
# and o = 1 - r needs no final max(., 0): far pixels (p underflowed) hit
# z = C -> s2 = ln 1 = 0 -> r = 1 -> o = 0, matching the reference's clip.
LN_SCALE_E = 60
LN_OFF = LN_SCALE_E * float(np.log(2.0))      # 42.2804
T_CLAMP = 41.5625          # fp16-exact; z >= LN_OFF - T_CLAMP = 0.027 > 0
T_FLOOR = LN_OFF - C_SOFT  # -43.7196; t >= T_FLOOR <=> z <= C_SOFT

_prog_cache = {}


# ---------------------------------------------------------------------------
# host-side math
# ---------------------------------------------------------------------------

def _bezier_samples(inputs_np):
    """[B,S,2] float64 sample points (y, x) in pixel coords."""
    kp = inputs_np.astype(np.float64) * np.array([H, W], np.float64)
    K = kp.shape[1]
    ts = np.linspace(0.0, 1.0, S)
    k = np.arange(K)
    binom = np.array([comb(K - 1, i) for i in range(K)], np.float64)
    basis = binom * ts[:, None] ** k * (1.0 - ts[:, None]) ** (K - 1 - k)
    return np.einsum("sk,bkd->bsd", basis, kp)


def _make_core_inputs(sp, widths, aas, core):
    """Input tensors for one core (curves [8*core, 8*core+8))."""
    import ml_dtypes

    bf16 = ml_dtypes.bfloat16
    coords = np.arange(256, dtype=np.float64)
    # one K=30 matmul per curve: rows (h, s); lhsT[30, 128] = U split by
    # x-half, rhs[30, 512] block-diagonal over (h, y) with V in each block
    ut = np.zeros((2 * S, CPB * 128), np.float32)
    vt = np.zeros((2 * S, CPB * 512), np.float32)
    at = np.zeros((128, UNITS * 256), np.float32)  # aa/2 per curve block
    with np.errstate(under="ignore"):
        for cl in range(CPB):
            b = core * CPB + cl
            kb = C_SOFT / float(widths[b]) ** 2
            sy, sx = sp[b, :, 0], sp[b, :, 1]
            U = np.exp(-kb * (coords[None, :] - sx[:, None]) ** 2)  # [S,256]
            V = np.exp(-kb * (coords[None, :] - sy[:, None]) ** 2)  # [S,256]
            for h in range(2):
                ut[h * S:(h + 1) * S, cl * 128:(cl + 1) * 128] = \
                    U[:, h * 128:(h + 1) * 128]
                vt[h * S:(h + 1) * S, cl * 512 + h * 256:cl * 512 + (h + 1) * 256] = V
            at[:, cl * 512:(cl + 1) * 512] = np.float32(aas[b] / 2.0)
    ut[ut < FLUSH] = 0.0
    vt[vt < FLUSH] = 0.0
    utb = ut.astype(bf16)
    vtb = vt.astype(bf16)
    # flush any bf16 denormals produced by the rounding itself
    utb[utb.astype(np.float32) < FLUSH] = 0
    vtb[vtb.astype(np.float32) < FLUSH] = 0
    bias = np.full((128, 1), LN_OFF / C_SOFT, np.float32)
    return {"ut": utb, "vt": vtb, "at": at.astype(bf16), "bias": bias}


# ---------------------------------------------------------------------------
# multi-wait workaround
# ---------------------------------------------------------------------------

def _split_multi_waits(nc):
    """This walrus build accepts only one sync-wait per instruction.  Hoist
    extra waits onto same-engine nops inserted just before the instruction
    (engine program order makes this semantically identical: all waits retire
    before the instruction issues)."""
    import concourse.mybir as mybir

    n = 0
    for fn in nc.m.functions:
        for bb in fn.blocks:
            insts = list(bb.instructions)
            out = []
            changed = False
            for inst in insts:
                si = inst.sync_info
                if si is not None and len(si.on_wait) > 1:
                    waits = list(si.on_wait)
                    for i, w in enumerate(waits[:-1]):
                        nop = mybir.InstNoOp(name=f"{inst.name}_xw{i}")
                        nop.engine = inst.engine
                        nop.sync_info = mybir.SyncInfo(on_wait=[w], on_update=[])
                        out.append(nop)
                        n += 1
                    inst.sync_info = mybir.SyncInfo(
                        on_wait=[waits[-1]], on_update=list(si.on_update)
                    )
                    changed = True
                out.append(inst)
            if changed:
                bb.instructions = out
    return n


# ---------------------------------------------------------------------------
# bass program (input-independent structure)
# ---------------------------------------------------------------------------

def _build_program(repeat=1, num_devices=NCORES):
    import concourse.bass as bass
    import concourse.mybir as mybir
    from concourse.tile import TileContext

    fp32 = mybir.dt.float32
    fp16 = mybir.dt.float16
    bf16 = mybir.dt.bfloat16
    A = mybir.AluOpType
    F = mybir.ActivationFunctionType

    nc = bass.Bass("TRN2", target_bir_lowering=False, debug=False,
                   num_devices=num_devices)
    ut_d = nc.dram_tensor("ut", [2 * S, CPB * 128], bf16, kind="ExternalInput")
    vt_d = nc.dram_tensor("vt", [2 * S, CPB * 512], bf16, kind="ExternalInput")
    at_d = nc.dram_tensor("at", [128, UNITS * 256], bf16, kind="ExternalInput")
    bias_d = nc.dram_tensor("bias", [128, 1], fp32, kind="ExternalInput")
    out_d = nc.dram_tensor("out", [128, UNITS * 256], bf16,
                           kind="ExternalOutput")

    HCURVE = CPB // 2          # curves per half-canvas chunk
    HCOL = HCURVE * 512        # 2048 columns per chunk

    with TileContext(nc) as tc:
        with (
            tc.tile_pool(name="const", bufs=1) as constp,
            tc.tile_pool(name="tail", bufs=2) as tailp,
            tc.tile_pool(name="ot", bufs=2) as otp,
            tc.psum_pool(name="psum", bufs=1) as psp,
        ):
            ut = constp.tile([2 * S, CPB * 128], bf16, tag="ut")
            vt = constp.tile([2 * S, CPB * 512], bf16, tag="vt")
            # split by half so the first matmuls start after half the bytes
            for h in range(2):
                nc.sync.dma_start(
                    out=ut[:, h * HCURVE * 128:(h + 1) * HCURVE * 128],
                    in_=ut_d[:, h * HCURVE * 128:(h + 1) * HCURVE * 128])
                nc.sync.dma_start(
                    out=vt[:, h * HCOL:(h + 1) * HCOL],
                    in_=vt_d[:, h * HCOL:(h + 1) * HCOL])
            bias = constp.tile([128, 1], fp32, tag="bias")
            nc.sync.dma_start(out=bias[:], in_=bias_d[:])
            at = constp.tile([128, UNITS * 256], bf16, tag="at")
            nc.sync.dma_start(out=at[:], in_=at_d[:])

            pp = psp.tile([128, UNITS * 256], fp32, tag="pp")

            def body():
                t = tailp.tile([128, UNITS * 256], fp32, tag="t")
                t2 = tailp.tile([128, UNITS * 256], fp32, tag="t2")
                s2 = tailp.tile([128, UNITS * 256], bf16, tag="s2")
                uu = tailp.tile([128, UNITS * 256], bf16, tag="uu")
                rr = tailp.tile([128, UNITS * 256], bf16, tag="rr")
                oo = otp.tile([128, UNITS * 256], bf16, tag="oo")

                # p = sum_s U[s,i] V[s,j]; one K=30 matmul per curve = bank
                for cl in range(CPB):
                    nc.tensor.matmul(
                        pp[:, cl * 512:(cl + 1) * 512],
                        ut[:, cl * 128:(cl + 1) * 128],
                        vt[:, cl * 512:(cl + 1) * 512],
                        start=True, stop=True,
                    )
                # t = ln p + LN_OFF   (fp32)
                nc.scalar.activation(t[:], pp[:], F.Ln,
                                     scale=float(2.0 ** LN_SCALE_E))
                # t2 = clamp(t): z = LN_OFF - t2 in [0.027, C] => md2 <= w^2
                nc.vector.tensor_scalar(t2[:], t[:], T_CLAMP,
                                        T_FLOOR, A.min, A.max)
                # s2 = ln((LN_OFF - t2)/C) = ln(z/C) = ln(md2/w^2)
                nc.scalar.activation(s2[:], t2[:], F.Ln,
                                     scale=-1.0 / C_SOFT, bias=bias[:])
                # u = (aa/2) ln(md2/w^2)
                nc.vector.tensor_tensor(uu[:], s2[:], at[:], A.mult)
                # r = (md/w)^aa
                nc.scalar.activation(rr[:], uu[:], F.Exp)
                # o = 1 - r  (r <= 1 by the z clamp: no final max needed)
                nc.vector.tensor_scalar(oo[:], rr[:], -1.0, 1.0, A.mult, A.add)
                nc.sync.dma_start(out=out_d[:], in_=oo[:])

            for _ in range(repeat):
                body()
    _split_multi_waits(nc)
    return nc


# ---------------------------------------------------------------------------
# public entry point
# ---------------------------------------------------------------------------

def _run(inputs, widths, aa_factors, repeat=1):
    from concourse.bass_utils import run_bass_kernel_spmd

    inputs = np.asarray(inputs, np.float32)
    widths = np.asarray(widths, np.float32)
    aa_factors = np.asarray(aa_factors, np.float32)
    assert inputs.shape == (B, 4, 2), inputs.shape

    sp = _bezier_samples(inputs)
    if repeat not in _prog_cache:
        _prog_cache[repeat] = _build_program(repeat)
    nc = _prog_cache[repeat]

    in_maps = [
        _make_core_inputs(sp, widths, aa_factors, c) for c in range(NCORES)
    ]
    try:
        res = run_bass_kernel_spmd(nc, in_maps, list(range(NCORES)))
    except Exception:
        # transient NRT/device hiccups recover on a fresh dispatch
        import time
        time.sleep(5.0)
        res = run_bass_kernel_spmd(nc, in_maps, list(range(NCORES)))

    canvas = np.empty((B, H, W), np.float32)
    for c in range(NCORES):
        out = np.asarray(res.results[c]["out"])          # [128, 4096] fp16
        out = out.reshape(128, CPB, 2, 256)              # [i, cl, h, y]
        out = out.transpose(1, 2, 0, 3)                  # [cl, h, i, y]
        canvas[c * CPB:(c + 1) * CPB] = out.reshape(CPB, 256, 256)
    return canvas


def kernel(inputs, widths, aa_factors):
    return _run(inputs, widths, aa_factors, repeat=1)

